# revision 24
# baseline (speedup 1.0000x reference)
"""Trainium2 Bass kernel for DigitConvolutionalModel.

Math: the 3x3 valid conv on the 28x28 image is a linear map, so it folds into
the first Linear layer:
    out = relu(x @ W_eff + b1) @ w2.T + b2
where W_eff[784, 128] = C @ w1.T and C[784, 676] is the conv-as-matrix built
from conv_w.  W_eff is built on the host (O(1) w.r.t. batch); the device does
the two batch matmuls.

Distribution: pure data parallel — batch dim of x sharded across 8 NeuronCores,
weights replicated.  Each core computes out.T [10, 8192]; the host reassembles
[65536, 10].

dtypes: x ships as float8e3 (e3m4: 4 mantissa bits), scaled by 2 on the host so
N(0,1) data sits in the normal range; the 1/2 is folded into the fp16 weights.
The PE multiplies fp16 weights by fp8 moving data directly (mixed operands are
supported); measured end-to-end rel-max error 1.28e-2 vs the 2e-2 gate.  fp8
halves HBM traffic to ~6.4 MB/core so the DMA stream stays ahead of the PE —
critical, since any PE idle gap makes the HAM down-clock the 2.4 GHz PE array
to half duty for ~7-14us.

Schedule: k-outer across 2 passes of 8 batch tiles.  All 8 PSUM banks act as
accumulators for one pass (bank t <- batch tile t); each of the 6 main k-tile
weights serves 8 consecutive matmuls.  The 16 remainder features
(784 = 6*128 + 16) are one K=16 accumulation matmul per tile, then a phased
epilogue: all 8 relus(+b1) split across ACT and DVE (so the relu chain drains
at ~365ns/tile and never stalls the PE's second-layer matmuls), 8 matmuls
[10,512] into the just-freed PSUM banks, then +b2 alternating ACT/DVE with a
store every two tiles — the final tail-critical store is only 4KB.

Rings: one HW DGE ring sustains only ~250-300 GB/s, marginal against the PE's
consumption rate, and the rings have an ~8us cold start.  x blocks 0..9 stream
on the sync ring; the last two (pass B, k=4,5) ride the scalar ring behind the
params, landing ~16us — removing the end-of-stream starvation that otherwise
stalls the PE at ~45us and triggers a half-clock HAM window over the tail;
block 3 also rides it to ease the sync ring's cold-start pacing.  The first
block transfers in halves so the PE starts ~1us earlier, and warm-up matmuls
on a memset tile (no DMA dependency) keep the PE busy from queue boot until
it lands, so the HAM ramp is never reset.
"""

import numpy as np
import ml_dtypes

import concourse.bass as bass  # noqa: F401  (bass registers mybir lowerings)
import concourse.mybir as mybir
import concourse.tile as tile
from concourse import bacc
from concourse.bass_utils import run_bass_kernel_spmd

N_CORES = 8
B = 65536
B_SH = B // N_CORES  # 8192 rows per core
D = 784              # 28*28 input features
DM = 768             # features in the main 128-partition stream
DR = D - DM          # 16 remainder features
H = 128              # hidden
OUT = 10
KT = 128             # contraction tile = full partition dim
NK = DM // KT        # 6 main K-tiles
NB = 512             # batch columns per tile (= one fp32 PSUM bank)
NP = 2               # passes
TPP = 8              # batch tiles per pass (= PSUM banks)
NWARM = 5            # PE clock warm-up matmuls (bridge queue boot -> first x piece)
N_SCALAR_X = 2       # trailing x blocks routed via the scalar ring

_CACHE = {}


def _strip_redundant_ldweights(nc, keep_every=4):
    """Drop some back-to-back InstLdweights with identical operands.

    Legalization emits one LDWEIGHTS per matmul even when consecutive
    matmuls share the stationary operand; the ISA matmul uses the
    currently-loaded weights.  Keep one LDW in every `keep_every` as
    insurance, plus any carrying a semaphore wait or with dependents.
    """
    dep_names = set()
    for f in nc.m.functions:
        for b in f.blocks:
            for i in b.instructions:
                dep_names.update(i.sync_dependency_names())
                dep_names.update(i.nosync_dependency_names())
    n_drop = 0
    for f in nc.m.functions:
        for b in f.blocks:
            insts = list(b.instructions)
            keep = []
            last_sig = None
            dropped_in_run = 0
            for i in insts:
                if type(i).__name__ == 'InstLdweights':
                    c = i.concise if isinstance(i.concise, str) else i.concise()
                    sig = c.split('in=', 1)[1] if 'in=' in c else None
                    if (sig is not None and sig == last_sig
                            and dropped_in_run < keep_every - 1
                            and 'wait:' not in c and i.name not in dep_names):
                        n_drop += 1
                        dropped_in_run += 1
                        continue
                    last_sig = sig
                    dropped_in_run = 0
                keep.append(i)
            if len(keep) != len(insts):
                b.instructions = keep
    return n_drop


def _build_nc():
    f32 = mybir.dt.float32
    f16 = mybir.dt.float16
    f8 = mybir.dt.float8e3
    nc = bacc.Bacc("TRN2", target_bir_lowering=False, debug=False,
                   num_devices=N_CORES)
    # main x, partition-major: [p, pass, k, t, c]; feature f = k*128 + p,
    # batch b = pass*4096 + t*512 + c.  Per (pass,k) DMA: 4 KB/partition runs.
    xk = nc.dram_tensor("xk", [KT, NP, NK, TPP, NB], f8,
                        kind="ExternalInput").ap()
    # remainder features 768..784: [p, batch]
    xrem = nc.dram_tensor("xrem", [DR, B_SH], f8, kind="ExternalInput").ap()
    # weights pre-arranged host-side: wk[p, k, m] = W_eff[k*128+p, m] / 2
    wk = nc.dram_tensor("wk", [KT, NK, H], f16, kind="ExternalInput").ap()
    wr = nc.dram_tensor("wr", [DR, H], f16, kind="ExternalInput").ap()
    w2t = nc.dram_tensor("w2t", [H, OUT], f16, kind="ExternalInput").ap()
    b1c = nc.dram_tensor("b1c", [H, 1], f32, kind="ExternalInput").ap()
    b2c = nc.dram_tensor("b2c", [OUT, 1], f32, kind="ExternalInput").ap()
    out = nc.dram_tensor("out", [OUT, B_SH], f16, kind="ExternalOutput").ap()

    NX = NP * NK

    with tile.TileContext(nc) as tc:
        with (
            tc.tile_pool(name="wpool", bufs=1) as wpool,
            tc.tile_pool(name="xpool", bufs=NX) as xpool,
            tc.tile_pool(name="hpool", bufs=6) as hpool,
            tc.tile_pool(name="opool", bufs=NP) as opool,
            tc.tile_pool(name="ps", bufs=8, space="PSUM") as pspool,
        ):
            # Params + remainder features + the last two x blocks ride the
            # scalar ring; the main x stream runs on the sync ring so its
            # first trigger issues at queue boot.
            wk_sb = wpool.tile([KT, NK, H], f16)
            nc.scalar.dma_start(wk_sb[:], wk[:])
            wr_sb = wpool.tile([DR, H], f16)
            nc.scalar.dma_start(wr_sb[:], wr[:])
            w2_sb = wpool.tile([H, OUT], f16)
            nc.scalar.dma_start(w2_sb[:], w2t[:])
            b1_sb = wpool.tile([H, 1], f32)
            nc.scalar.dma_start(b1_sb[:], b1c[:])
            b2_sb = wpool.tile([OUT, 1], f32)
            nc.scalar.dma_start(b2_sb[:], b2c[:])
            xr_sb = wpool.tile([DR, B_SH], f8)
            nc.scalar.dma_start(xr_sb[:], xrem[:])

            # x blocks, emitted in consumption order; all 12 resident in SBUF
            # (no buffer-reuse stalls).  Blocks 0..9 on the sync ring; the
            # last two on the scalar ring (behind the params) land ~16us,
            # long before the PE reaches them at ~38us.
            x_sb = []
            for pa in range(NP):
                for k in range(NK):
                    i = pa * NK + k
                    t_ = xpool.tile([KT, TPP * NB], f8, name="xb")
                    eng = nc.scalar if i in (3, 5, 7, 10, 11) else nc.sync
                    if i == 0:
                        # first block laddered (1,1,2,4 tiles): the PE starts
                        # on the first 64KB piece ~2us earlier and ramps while
                        # the rest streams in
                        for lo, hi in ((0, 1), (1, 2), (2, 4), (4, TPP)):
                            eng.dma_start(
                                t_[:, lo * NB:hi * NB],
                                xk[:, 0, 0, lo:hi, :].rearrange(
                                    "p t c -> p (t c)"))
                    elif i == 1:
                        # second block in halves: smooths the k0->k1 handoff
                        half = TPP // 2
                        eng.dma_start(
                            t_[:, :half * NB],
                            xk[:, 0, 1, :half, :].rearrange("p t c -> p (t c)"))
                        eng.dma_start(
                            t_[:, half * NB:],
                            xk[:, 0, 1, half:, :].rearrange("p t c -> p (t c)"))
                    else:
                        eng.dma_start(
                            t_[:], xk[:, pa, k, :, :].rearrange("p t c -> p (t c)"))
                    x_sb.append(t_)

            # PE clock warm-up: HAM reaches 2.4 GHz after ~3.4us of activity;
            # these run from queue boot while the first x block is in flight.
            warm_x = wpool.tile([KT, NB], f16)
            # gpsimd memset: its queue boots first (it runs the framework's
            # const-init memsets), so warm-up starts ~1.3us earlier than via
            # the DVE and full clock arrives before the first real matmul
            nc.gpsimd.memset(warm_x[:], 0.0)
            warm_ps = pspool.tile([H, NB], f32, name="acc")
            for _ in range(NWARM):
                nc.tensor.matmul(warm_ps[:], lhsT=warm_x[:, 0:H],
                                 rhs=warm_x[:], start=True, stop=True)

            for pa in range(NP):
                ps_t = [pspool.tile([H, NB], f32, name="acc")
                        for t in range(TPP)]
                # k-outer: one stationary weight feeds 8 consecutive matmuls
                for k in range(NK):
                    xs = x_sb[pa * NK + k]
                    for t in range(TPP):
                        nc.tensor.matmul(
                            ps_t[t][:],
                            lhsT=wk_sb[:, k, :],
                            rhs=xs[:, t * NB:(t + 1) * NB],
                            start=(k == 0),
                            stop=False,
                        )
                # batched remainder matmuls (one wr weight load); relu(t)
                # starts as rem(t) completes, so the h tiles are ready by the
                # time the batched second-layer matmuls issue
                for t in range(TPP):
                    b0 = (pa * TPP + t) * NB
                    nc.tensor.matmul(
                        ps_t[t][:], lhsT=wr_sb[:],
                        rhs=xr_sb[:, b0:b0 + NB],
                        start=False, stop=True,
                    )
                o_sb = opool.tile([OUT, TPP * NB], f16)
                # phase 1: all relus, split across ACT and DVE, so the 8-tile
                # relu chain drains at ~365ns/tile and relu(7) is ready ~2us
                # before the PE's last L2 matmul needs it
                h_t = []
                for t in range(TPP):
                    h_sb = hpool.tile([H, NB], f16, name="hb")
                    if t % 2 == 0:
                        nc.scalar.activation(
                            h_sb[:], ps_t[t][:],
                            mybir.ActivationFunctionType.Relu, bias=b1_sb[:])
                    else:
                        nc.vector.tensor_scalar(
                            h_sb[:], ps_t[t][:], b1_sb[:], 0.0,
                            mybir.AluOpType.add, mybir.AluOpType.max)
                    h_t.append(h_sb)
                # phase 2: second-layer matmuls into the just-freed banks
                ps2_t = []
                for t in range(TPP):
                    ps2 = pspool.tile([OUT, NB], f32, name="acc")
                    nc.tensor.matmul(ps2[:], lhsT=w2_sb[:], rhs=h_t[t][:],
                                     start=True, stop=True)
                    ps2_t.append(ps2)
                # phase 3: +b2 (behind the relus on each queue) and stores
                # every two tiles, keeping the final tail store small
                for t in range(TPP):
                    if t % 2 == 0:
                        nc.vector.tensor_scalar_add(
                            o_sb[:, t * NB:(t + 1) * NB], ps2_t[t][:],
                            b2_sb[:])  # fp16 out
                    else:
                        nc.scalar.activation(
                            o_sb[:, t * NB:(t + 1) * NB], ps2_t[t][:],
                            mybir.ActivationFunctionType.Identity,
                            bias=b2_sb[:])
                    if t % 2 == 1:
                        b0 = (pa * TPP + t - 1) * NB
                        nc.sync.dma_start(
                            out[:, b0:b0 + 2 * NB],
                            o_sb[:, (t - 1) * NB:(t + 1) * NB])

    nc.compile()
    import os
    if os.environ.get("STRIP_LDW", "0") == "1":
        _strip_redundant_ldweights(nc)
    return nc


def _get_nc():
    if "nc" not in _CACHE:
        _CACHE["nc"] = _build_nc()
    return _CACHE["nc"]


def _fold_weights(conv_w: np.ndarray, w1: np.ndarray) -> np.ndarray:
    """W_eff[784, 128]: h_pre = x @ W_eff  ==  conv(x) @ w1.T  (float64 accum)."""
    w1k = w1.reshape(H, 26, 26).transpose(1, 2, 0).astype(np.float64)  # [i,j,k]
    cw = conv_w.astype(np.float64)
    W = np.zeros((28, 28, H), np.float64)
    for di in range(3):
        for dj in range(3):
            W[di:di + 26, dj:dj + 26, :] += cw[di, dj] * w1k
    return W.reshape(D, H).astype(np.float32)


def make_in_maps(x, conv_w, w1, b1, w2, b2):
    f8 = ml_dtypes.float8_e3m4
    x = np.asarray(x, np.float32)
    weff = _fold_weights(np.asarray(conv_w, np.float32),
                         np.asarray(w1, np.float32)) * 0.5  # absorb x*2
    # wk[p, k, m] = weff[k*128+p, m]
    wk = np.ascontiguousarray(
        weff[:DM].reshape(NK, KT, H).transpose(1, 0, 2)).astype(np.float16)
    wr = np.ascontiguousarray(weff[DM:]).astype(np.float16)
    w2t = np.ascontiguousarray(np.asarray(w2, np.float32).T).astype(np.float16)
    b1c = np.ascontiguousarray(np.asarray(b1, np.float32).reshape(H, 1))
    b2c = np.ascontiguousarray(np.asarray(b2, np.float32).reshape(OUT, 1))
    in_maps = []
    for i in range(N_CORES):
        xs = (x[i * B_SH:(i + 1) * B_SH] * 2.0).astype(f8)  # [8192, 784]
        # main: [pass*4096 + t*512 + c, k*128 + p] -> [p, pass, k, t, c]
        xkv = np.ascontiguousarray(
            xs[:, :DM].reshape(NP, TPP, NB, NK, KT).transpose(4, 0, 3, 1, 2))
        xremv = np.ascontiguousarray(xs[:, DM:].T)           # [16, 8192]
        in_maps.append({"xk": xkv, "xrem": xremv, "wk": wk, "wr": wr,
                        "w2t": w2t, "b1c": b1c, "b2c": b2c})
    return in_maps


def kernel(x, conv_w, w1, b1, w2, b2):
    nc = _get_nc()
    in_maps = make_in_maps(x, conv_w, w1, b1, w2, b2)
    res = run_bass_kernel_spmd(nc, in_maps, list(range(N_CORES)))
    out = np.concatenate([res.results[i]["out"] for i in range(N_CORES)], axis=1)
    return np.ascontiguousarray(out.T.astype(np.float32))  # [65536, 10]


# revision 25
# speedup vs baseline: 1.0491x; 1.0491x over previous
"""Trainium2 Bass kernel for DigitConvolutionalModel.

Math: the 3x3 valid conv on the 28x28 image is a linear map, so it folds into
the first Linear layer:
    out = relu(x @ W_eff + b1) @ w2.T + b2
where W_eff[784, 128] = C @ w1.T and C[784, 676] is the conv-as-matrix built
from conv_w.  W_eff is built on the host (O(1) w.r.t. batch); the device does
the two batch matmuls.

Distribution: pure data parallel — batch dim of x sharded across 8 NeuronCores,
weights replicated.  Each core computes out.T [10, 8192]; the host reassembles
[65536, 10].

dtypes: x ships as float8e3 (e3m4: 4 mantissa bits), scaled by 2 on the host so
N(0,1) data sits in the normal range; the 1/2 is folded into the fp16 weights.
The PE multiplies fp16 weights by fp8 moving data directly (mixed operands are
supported); measured end-to-end rel-max error 1.28e-2 vs the 2e-2 gate.  fp8
halves HBM traffic to ~6.4 MB/core so the DMA stream stays ahead of the PE —
critical, since any PE idle gap makes the HAM down-clock the 2.4 GHz PE array
to half duty for ~7-14us.

Schedule: k-outer across 2 passes of 8 batch tiles.  All 8 PSUM banks act as
accumulators for one pass (bank t <- batch tile t); each of the 6 main k-tile
weights serves 8 consecutive matmuls.  The 16 remainder features
(784 = 6*128 + 16) are one K=16 accumulation matmul per tile, then a phased
epilogue: all 8 relus(+b1) split across ACT and DVE (so the relu chain drains
at ~365ns/tile and never stalls the PE's second-layer matmuls), 8 matmuls
[10,512] into the just-freed PSUM banks, then +b2 alternating ACT/DVE with a
store every two tiles — the final tail-critical store is only 4KB.

Rings: one HW DGE ring sustains only ~250-300 GB/s, marginal against the PE's
consumption rate, and the rings have an ~8us cold start.  x blocks 0..9 stream
on the sync ring; the last two (pass B, k=4,5) ride the scalar ring behind the
params, landing ~16us — removing the end-of-stream starvation that otherwise
stalls the PE at ~45us and triggers a half-clock HAM window over the tail;
block 3 also rides it to ease the sync ring's cold-start pacing.  The first
block transfers in halves so the PE starts ~1us earlier, and warm-up matmuls
on a memset tile (no DMA dependency) keep the PE busy from queue boot until
it lands, so the HAM ramp is never reset.
"""

import numpy as np
import ml_dtypes

import concourse.bass as bass  # noqa: F401  (bass registers mybir lowerings)
import concourse.mybir as mybir
import concourse.tile as tile
from concourse import bacc
from concourse.bass_utils import run_bass_kernel_spmd

N_CORES = 8
B = 65536
B_SH = B // N_CORES  # 8192 rows per core
D = 784              # 28*28 input features
DM = 768             # features in the main 128-partition stream
DR = D - DM          # 16 remainder features
H = 128              # hidden
OUT = 10
KT = 128             # contraction tile = full partition dim
NK = DM // KT        # 6 main K-tiles
NB = 512             # batch columns per tile (= one fp32 PSUM bank)
NP = 2               # passes
TPP = 8              # batch tiles per pass (= PSUM banks)
NWARM = 7            # PE warm-up matmuls: bridge queue boot -> first x piece
                     # even when ring delivery is slow (~11.4us worst observed)
N_SCALAR_X = 2       # trailing x blocks routed via the scalar ring

_CACHE = {}


def _strip_redundant_ldweights(nc, keep_every=4):
    """Drop some back-to-back InstLdweights with identical operands.

    Legalization emits one LDWEIGHTS per matmul even when consecutive
    matmuls share the stationary operand; the ISA matmul uses the
    currently-loaded weights.  Keep one LDW in every `keep_every` as
    insurance, plus any carrying a semaphore wait or with dependents.
    """
    dep_names = set()
    for f in nc.m.functions:
        for b in f.blocks:
            for i in b.instructions:
                dep_names.update(i.sync_dependency_names())
                dep_names.update(i.nosync_dependency_names())
    n_drop = 0
    for f in nc.m.functions:
        for b in f.blocks:
            insts = list(b.instructions)
            keep = []
            last_sig = None
            dropped_in_run = 0
            for i in insts:
                if type(i).__name__ == 'InstLdweights':
                    c = i.concise if isinstance(i.concise, str) else i.concise()
                    sig = c.split('in=', 1)[1] if 'in=' in c else None
                    if (sig is not None and sig == last_sig
                            and dropped_in_run < keep_every - 1
                            and 'wait:' not in c and i.name not in dep_names):
                        n_drop += 1
                        dropped_in_run += 1
                        continue
                    last_sig = sig
                    dropped_in_run = 0
                keep.append(i)
            if len(keep) != len(insts):
                b.instructions = keep
    return n_drop


def _build_nc():
    f32 = mybir.dt.float32
    f16 = mybir.dt.float16
    f8 = mybir.dt.float8e3
    nc = bacc.Bacc("TRN2", target_bir_lowering=False, debug=False,
                   num_devices=N_CORES)
    # main x, partition-major: [p, pass, k, t, c]; feature f = k*128 + p,
    # batch b = pass*4096 + t*512 + c.  Per (pass,k) DMA: 4 KB/partition runs.
    xk = nc.dram_tensor("xk", [KT, NP, NK, TPP, NB], f8,
                        kind="ExternalInput").ap()
    # remainder features 768..784: [p, batch]
    xrem = nc.dram_tensor("xrem", [DR, B_SH], f8, kind="ExternalInput").ap()
    # weights pre-arranged host-side: wk[p, k, m] = W_eff[k*128+p, m] / 2
    wk = nc.dram_tensor("wk", [KT, NK, H], f16, kind="ExternalInput").ap()
    wr = nc.dram_tensor("wr", [DR, H], f16, kind="ExternalInput").ap()
    w2t = nc.dram_tensor("w2t", [H, OUT], f16, kind="ExternalInput").ap()
    b1c = nc.dram_tensor("b1c", [H, 1], f32, kind="ExternalInput").ap()
    b2c = nc.dram_tensor("b2c", [OUT, 1], f32, kind="ExternalInput").ap()
    out = nc.dram_tensor("out", [OUT, B_SH], f16, kind="ExternalOutput").ap()

    NX = NP * NK

    with tile.TileContext(nc) as tc:
        with (
            tc.tile_pool(name="wpool", bufs=1) as wpool,
            tc.tile_pool(name="xpool", bufs=NX) as xpool,
            tc.tile_pool(name="hpool", bufs=6) as hpool,
            tc.tile_pool(name="opool", bufs=NP) as opool,
            tc.tile_pool(name="ps", bufs=8, space="PSUM") as pspool,
        ):
            # Params + remainder features + the last two x blocks ride the
            # scalar ring; the main x stream runs on the sync ring so its
            # first trigger issues at queue boot.
            wk_sb = wpool.tile([KT, NK, H], f16)
            nc.scalar.dma_start(wk_sb[:], wk[:])
            wr_sb = wpool.tile([DR, H], f16)
            nc.scalar.dma_start(wr_sb[:], wr[:])
            w2_sb = wpool.tile([H, OUT], f16)
            nc.scalar.dma_start(w2_sb[:], w2t[:])
            b1_sb = wpool.tile([H, 1], f32)
            nc.scalar.dma_start(b1_sb[:], b1c[:])
            b2_sb = wpool.tile([OUT, 1], f32)
            nc.scalar.dma_start(b2_sb[:], b2c[:])
            xr_sb = wpool.tile([DR, B_SH], f8)
            nc.scalar.dma_start(xr_sb[:], xrem[:])

            # x blocks, emitted in consumption order; all 12 resident in SBUF
            # (no buffer-reuse stalls).  Blocks 0..9 on the sync ring; the
            # last two on the scalar ring (behind the params) land ~16us,
            # long before the PE reaches them at ~38us.
            x_sb = []
            for pa in range(NP):
                for k in range(NK):
                    i = pa * NK + k
                    t_ = xpool.tile([KT, TPP * NB], f8, name="xb")
                    eng = nc.scalar if i in (3, 5, 7, 10, 11) else nc.sync
                    if i == 0:
                        # first block laddered (1,1,2,4 tiles): the PE starts
                        # on the first 64KB piece ~2us earlier and ramps while
                        # the rest streams in
                        for lo, hi in ((0, 1), (1, 2), (2, 4), (4, TPP)):
                            eng.dma_start(
                                t_[:, lo * NB:hi * NB],
                                xk[:, 0, 0, lo:hi, :].rearrange(
                                    "p t c -> p (t c)"))
                    elif i == 1:
                        # second block in halves: smooths the k0->k1 handoff
                        half = TPP // 2
                        eng.dma_start(
                            t_[:, :half * NB],
                            xk[:, 0, 1, :half, :].rearrange("p t c -> p (t c)"))
                        eng.dma_start(
                            t_[:, half * NB:],
                            xk[:, 0, 1, half:, :].rearrange("p t c -> p (t c)"))
                    else:
                        eng.dma_start(
                            t_[:], xk[:, pa, k, :, :].rearrange("p t c -> p (t c)"))
                    x_sb.append(t_)

            # PE clock warm-up: HAM reaches 2.4 GHz after ~3.4us of activity;
            # these run from queue boot while the first x block is in flight.
            warm_x = wpool.tile([KT, NB], f16)
            # gpsimd memset: its queue boots first (it runs the framework's
            # const-init memsets), so warm-up starts ~1.3us earlier than via
            # the DVE and full clock arrives before the first real matmul
            nc.gpsimd.memset(warm_x[:], 0.0)
            warm_ps = pspool.tile([H, NB], f32, name="acc")
            for _ in range(NWARM):
                nc.tensor.matmul(warm_ps[:], lhsT=warm_x[:, 0:H],
                                 rhs=warm_x[:], start=True, stop=True)

            for pa in range(NP):
                ps_t = [pspool.tile([H, NB], f32, name="acc")
                        for t in range(TPP)]
                # k-outer: one stationary weight feeds 8 consecutive matmuls
                for k in range(NK):
                    xs = x_sb[pa * NK + k]
                    for t in range(TPP):
                        nc.tensor.matmul(
                            ps_t[t][:],
                            lhsT=wk_sb[:, k, :],
                            rhs=xs[:, t * NB:(t + 1) * NB],
                            start=(k == 0),
                            stop=False,
                        )
                # batched remainder matmuls (one wr weight load); relu(t)
                # starts as rem(t) completes, so the h tiles are ready by the
                # time the batched second-layer matmuls issue
                for t in range(TPP):
                    b0 = (pa * TPP + t) * NB
                    nc.tensor.matmul(
                        ps_t[t][:], lhsT=wr_sb[:],
                        rhs=xr_sb[:, b0:b0 + NB],
                        start=False, stop=True,
                    )
                o_sb = opool.tile([OUT, TPP * NB], f16)
                # phase 1: all relus, split across ACT and DVE, so the 8-tile
                # relu chain drains at ~365ns/tile and relu(7) is ready ~2us
                # before the PE's last L2 matmul needs it
                h_t = []
                for t in range(TPP):
                    h_sb = hpool.tile([H, NB], f16, name="hb")
                    if t % 2 == 0:
                        nc.scalar.activation(
                            h_sb[:], ps_t[t][:],
                            mybir.ActivationFunctionType.Relu, bias=b1_sb[:])
                    else:
                        nc.vector.tensor_scalar(
                            h_sb[:], ps_t[t][:], b1_sb[:], 0.0,
                            mybir.AluOpType.add, mybir.AluOpType.max)
                    h_t.append(h_sb)
                # phase 2: second-layer matmuls into the just-freed banks
                ps2_t = []
                for t in range(TPP):
                    ps2 = pspool.tile([OUT, NB], f32, name="acc")
                    nc.tensor.matmul(ps2[:], lhsT=w2_sb[:], rhs=h_t[t][:],
                                     start=True, stop=True)
                    ps2_t.append(ps2)
                # phase 3: +b2 (behind the relus on each queue) and stores
                # every two tiles, keeping the final tail store small
                for t in range(TPP):
                    if t % 2 == 0:
                        nc.vector.tensor_scalar_add(
                            o_sb[:, t * NB:(t + 1) * NB], ps2_t[t][:],
                            b2_sb[:])  # fp16 out
                    else:
                        nc.scalar.activation(
                            o_sb[:, t * NB:(t + 1) * NB], ps2_t[t][:],
                            mybir.ActivationFunctionType.Identity,
                            bias=b2_sb[:])
                    if t % 2 == 1:
                        b0 = (pa * TPP + t - 1) * NB
                        nc.sync.dma_start(
                            out[:, b0:b0 + 2 * NB],
                            o_sb[:, (t - 1) * NB:(t + 1) * NB])

    nc.compile()
    import os
    if os.environ.get("STRIP_LDW", "0") == "1":
        _strip_redundant_ldweights(nc)
    return nc


def _get_nc():
    if "nc" not in _CACHE:
        _CACHE["nc"] = _build_nc()
    return _CACHE["nc"]


def _fold_weights(conv_w: np.ndarray, w1: np.ndarray) -> np.ndarray:
    """W_eff[784, 128]: h_pre = x @ W_eff  ==  conv(x) @ w1.T  (float64 accum)."""
    w1k = w1.reshape(H, 26, 26).transpose(1, 2, 0).astype(np.float64)  # [i,j,k]
    cw = conv_w.astype(np.float64)
    W = np.zeros((28, 28, H), np.float64)
    for di in range(3):
        for dj in range(3):
            W[di:di + 26, dj:dj + 26, :] += cw[di, dj] * w1k
    return W.reshape(D, H).astype(np.float32)


def make_in_maps(x, conv_w, w1, b1, w2, b2):
    f8 = ml_dtypes.float8_e3m4
    x = np.asarray(x, np.float32)
    weff = _fold_weights(np.asarray(conv_w, np.float32),
                         np.asarray(w1, np.float32)) * 0.5  # absorb x*2
    # wk[p, k, m] = weff[k*128+p, m]
    wk = np.ascontiguousarray(
        weff[:DM].reshape(NK, KT, H).transpose(1, 0, 2)).astype(np.float16)
    wr = np.ascontiguousarray(weff[DM:]).astype(np.float16)
    w2t = np.ascontiguousarray(np.asarray(w2, np.float32).T).astype(np.float16)
    b1c = np.ascontiguousarray(np.asarray(b1, np.float32).reshape(H, 1))
    b2c = np.ascontiguousarray(np.asarray(b2, np.float32).reshape(OUT, 1))
    in_maps = []
    for i in range(N_CORES):
        xs = (x[i * B_SH:(i + 1) * B_SH] * 2.0).astype(f8)  # [8192, 784]
        # main: [pass*4096 + t*512 + c, k*128 + p] -> [p, pass, k, t, c]
        xkv = np.ascontiguousarray(
            xs[:, :DM].reshape(NP, TPP, NB, NK, KT).transpose(4, 0, 3, 1, 2))
        xremv = np.ascontiguousarray(xs[:, DM:].T)           # [16, 8192]
        in_maps.append({"xk": xkv, "xrem": xremv, "wk": wk, "wr": wr,
                        "w2t": w2t, "b1c": b1c, "b2c": b2c})
    return in_maps


def kernel(x, conv_w, w1, b1, w2, b2):
    nc = _get_nc()
    in_maps = make_in_maps(x, conv_w, w1, b1, w2, b2)
    res = run_bass_kernel_spmd(nc, in_maps, list(range(N_CORES)))
    out = np.concatenate([res.results[i]["out"] for i in range(N_CORES)], axis=1)
    return np.ascontiguousarray(out.T.astype(np.float32))  # [65536, 10]
